# revision 22
# baseline (speedup 1.0000x reference)
"""Block-sparse (banded) attention kernel for Trainium2, 8 NeuronCores.

Sharding: data-parallel over batch (2) x tensor-parallel over heads
(16 heads -> 4 per core).  Each core computes its 4 heads' Q/K/V
projections, banded block attention (|r-c| <= 15 blocks, per-block
softmax), and a partial output projection; the host sums the 4 partial
outputs per batch element.

Pipeline (per head h, per chunk i of 2 query blocks = 128 q rows):
  scores[q, k] = qT^T @ kT on PE (q on partitions, banded k extent free)
  exp on ACT (psum f32 -> sbuf bf16)
  per-block denominators via DVE tensor_reduce (innermost-64 seg-sum)
  reciprocal + band mask on DVE
  normalize-mul with stride-0 broadcast AP (split DVE / gpsimd)
  transpose probs via DMA XBAR (dma_start_transpose, sbuf->sbuf bf16)
  AV matmul on PE (k on partitions), accumulate over k-block pairs
  output projection (head-pair K=128 packed) -> DMA partials from PSUM

Everything flows bf16 into the PE; host pre-casts inputs to bf16.
"""

import sys

for _p in ("/opt/trn_rl_repo",):
    if _p not in sys.path:
        sys.path.insert(0, _p)

from contextlib import ExitStack

import numpy as np
import ml_dtypes

import concourse.bacc as bacc
import concourse.tile as tile
from concourse import bass_utils, mybir

F32 = mybir.dt.float32
BF16 = mybir.dt.bfloat16
EXP = mybir.ActivationFunctionType.Exp

B, S, E = 2, 2048, 1024
H, HD, BLK = 16, 64, 64
NB = S // BLK  # 32 blocks
NCORES = 8
HPC = 4  # heads per core
F = HPC * HD  # 256 local features
BAND = 15
SCALE = HD ** -0.5
NCH = 16  # chunks per head: 2 query blocks (128 q rows) each

# per-chunk banded k-block range, even-aligned for the AV pair layout
LO = []
NBK = []
for _i in range(NCH):
    lo = max(0, 2 * _i - BAND)
    lo -= lo % 2
    hi = min(NB - 1, 2 * _i + 1 + BAND)
    if (hi - lo + 1) % 2 == 1:
        if hi < NB - 1:
            hi += 1
        else:
            lo -= 2
    LO.append(lo)
    NBK.append(hi - lo + 1)
# split each chunk into two even-block halves (each <= 16 blocks = 1024 k)
NBA = [((nb // 2 + 1) // 2) * 2 for nb in NBK]


def build_nc(debug=False):
    nc = bacc.Bacc("TRN2", target_bir_lowering=False, debug=False)

    xq_d = nc.dram_tensor("xqT", [E, S], BF16, kind="ExternalInput")
    xk_d = nc.dram_tensor("xkT", [E, S], BF16, kind="ExternalInput")
    xv_d = nc.dram_tensor("xvT", [E, S], BF16, kind="ExternalInput")
    wq_d = nc.dram_tensor("wqT", [E, F], BF16, kind="ExternalInput")
    wk_d = nc.dram_tensor("wkT", [E, F], BF16, kind="ExternalInput")
    wv_d = nc.dram_tensor("wvT", [E, F], BF16, kind="ExternalInput")
    wo_d = nc.dram_tensor("wo2", [128, 2 * E], BF16, kind="ExternalInput")
    mk_d = nc.dram_tensor("maskc", [128, NCH * 32], F32, kind="ExternalInput")
    out_d = nc.dram_tensor("out", [S, E], F32, kind="ExternalOutput")
    if debug:
        qT_dbg = nc.dram_tensor("qT_dbg", [128, 2 * S], BF16, kind="ExternalOutput")
        kT_dbg = nc.dram_tensor("kT_dbg", [128, 2 * S], BF16, kind="ExternalOutput")
        vv_dbg = nc.dram_tensor("vv_dbg", [128, 16 * F], BF16, kind="ExternalOutput")
        ex_dbg = nc.dram_tensor("ex_dbg", [128, 128 * 1024], BF16, kind="ExternalOutput")
        pt_dbg = nc.dram_tensor("pt_dbg", [128, 128 * 1024], BF16, kind="ExternalOutput")
        ptT_dbg = nc.dram_tensor("ptT_dbg", [128, 128 * 1024], BF16, kind="ExternalOutput")
        rc_dbg = nc.dram_tensor("rc_dbg", [128, 128 * 16], F32, kind="ExternalOutput")
        at_dbg = nc.dram_tensor("at_dbg", [128, 2 * S], BF16, kind="ExternalOutput")

    with tile.TileContext(nc) as tc, ExitStack() as ctx, nc.allow_low_precision(
        reason="bf16 pipeline; tolerance 2e-2"
    ):
        pers = ctx.enter_context(tc.tile_pool(name="pers", bufs=1))
        qT = pers.tile([128, 2 * S], BF16, tag="qT")
        kT = pers.tile([128, 2 * S], BF16, tag="kT")
        vv = pers.tile([128, 16 * F], BF16, tag="vv")
        wq = pers.tile([128, 8 * F], BF16, tag="wq")
        wk = pers.tile([128, 8 * F], BF16, tag="wk")
        wv = pers.tile([128, 8 * F], BF16, tag="wv")
        wo = pers.tile([128, 2 * E], BF16, tag="wo")
        mkc = pers.tile([128, NCH * 32], F32, tag="mkc")
        attT = [
            pers.tile([128, S], BF16, name=f"attT{p}", tag=f"attT{p}")
            for p in range(2)
        ]

        # k-projection weights first: phase 1 is on the critical path
        nc.sync.dma_start(
            wk[:].rearrange("p (c f) -> p c f", c=8),
            wk_d.ap().rearrange("(c p) f -> p c f", p=128),
        )
        nc.gpsimd.dma_start(
            wv[:].rearrange("p (c f) -> p c f", c=8),
            wv_d.ap().rearrange("(c p) f -> p c f", p=128),
        )
        nc.gpsimd.dma_start(
            wq[:].rearrange("p (c f) -> p c f", c=8),
            wq_d.ap().rearrange("(c p) f -> p c f", p=128),
        )
        nc.gpsimd.dma_start(wo[:], wo_d.ap())
        nc.gpsimd.dma_start(mkc[:], mk_d.ap())

        # ---- phase 1: k projection (kT layout [f, s]) ----
        with tc.tile_pool(name="xk", bufs=2) as xkp, tc.tile_pool(
            name="psK", bufs=1, space="PSUM"
        ) as pskp:
            psK = pskp.tile([128, 4096], F32)
            for e in range(8):
                xt = xkp.tile([128, S], BF16, tag="xk")
                nc.sync.dma_start(xt[:], xk_d.ap()[e * 128 : (e + 1) * 128, :])
                for fold in range(2):
                    for sc in range(4):
                        nc.tensor.matmul(
                            psK[:, (fold * 4 + sc) * 512 : (fold * 4 + sc + 1) * 512],
                            wk[:, e * F + fold * 128 : e * F + fold * 128 + 128],
                            xt[:, sc * 512 : (sc + 1) * 512],
                            start=(e == 0),
                            stop=(e == 7),
                        )
            for fold in range(2):
                for sc in range(4):
                    nc.scalar.copy(
                        kT[:, fold * S + sc * 512 : fold * S + (sc + 1) * 512],
                        psK[:, (fold * 4 + sc) * 512 : (fold * 4 + sc + 1) * 512],
                    )

        # ---- phase 2: v projection (vv layout [kpair*128, cp*F + h*64 + d]) ----
        with tc.tile_pool(name="xv", bufs=3) as xvp, tc.tile_pool(
            name="psV", bufs=2, space="PSUM"
        ) as psvp:
            for sc in range(4):
                pvs = [
                    psvp.tile([128, 256], F32, name=f"pv{sub}", tag=f"psV{sub}")
                    for sub in range(4)
                ]
                for e in range(8):
                    xt = xvp.tile([128, 512], BF16, tag="xv")
                    nc.sync.dma_start(
                        xt[:],
                        xv_d.ap()[e * 128 : (e + 1) * 128, sc * 512 : (sc + 1) * 512],
                    )
                    for sub in range(4):
                        nc.tensor.matmul(
                            pvs[sub][:],
                            xt[:, sub * 128 : (sub + 1) * 128],
                            wv[:, e * F : (e + 1) * F],
                            start=(e == 0),
                            stop=(e == 7),
                        )
                for sub in range(4):
                    nc.scalar.copy(
                        vv[:, sc * 1024 + sub * 256 : sc * 1024 + (sub + 1) * 256],
                        pvs[sub][:],
                    )

        # ---- phase 3: q projection (interleaved) + attention + out projection
        xqp = ctx.enter_context(tc.tile_pool(name="xq", bufs=3))
        psSp = ctx.enter_context(tc.tile_pool(name="psS", bufs=2, space="PSUM"))
        psAp = ctx.enter_context(tc.tile_pool(name="psA", bufs=2, space="PSUM"))
        psOp = ctx.enter_context(tc.tile_pool(name="psO", bufs=1, space="PSUM"))
        expp = ctx.enter_context(tc.tile_pool(name="expS", bufs=4))
        ptp = ctx.enter_context(tc.tile_pool(name="pt", bufs=3))
        ptTp = ctx.enter_context(tc.tile_pool(name="ptT", bufs=3))
        rcpp = ctx.enter_context(tc.tile_pool(name="rcp", bufs=4))
        outp = ctx.enter_context(tc.tile_pool(name="outsb", bufs=2))

        def qproj(sc4):
            ps = psSp.tile([128, 1024], F32, tag="psS")
            for e in range(8):
                xt = xqp.tile([128, 512], BF16, tag="xq")
                nc.sync.dma_start(
                    xt[:],
                    xq_d.ap()[e * 128 : (e + 1) * 128, sc4 * 512 : (sc4 + 1) * 512],
                )
                for fold in range(2):
                    nc.tensor.matmul(
                        ps[:, fold * 512 : (fold + 1) * 512],
                        wq[:, e * F + fold * 128 : e * F + fold * 128 + 128],
                        xt[:],
                        start=(e == 0),
                        stop=(e == 7),
                    )
            for fold in range(2):
                nc.scalar.copy(
                    qT[:, fold * S + sc4 * 512 : fold * S + (sc4 + 1) * 512],
                    ps[:, fold * 512 : (fold + 1) * 512],
                )

        nrmctr = [0]

        def unit(h, i):
            """One (head, chunk): 128 q rows x banded k extent."""
            nb, lo = NBK[i], LO[i]
            nba = NBA[i]
            fold = h // 2
            bp = 64 * (h % 2)
            qsl = slice(fold * S + 128 * i, fold * S + 128 * i + 128)
            acc = psAp.tile([128, 512], F32, tag="psA")
            WF = nb * 64
            pt = ptp.tile([128, 2048], BF16, tag="pt")
            ptT = ptTp.tile([128, 2048], BF16, tag="ptT")
            for half in range(2):
                nbh = nba if half == 0 else nb - nba
                ho = half * nba * 64
                ko = lo * 64 + ho
                W = nbh * 64
                ps = psSp.tile([128, 1024], F32, tag="psS")
                off = 0
                while off < W:
                    n = min(512, W - off)
                    nc.tensor.matmul(
                        ps[:, off : off + n],
                        qT[bp : bp + 64, qsl],
                        kT[bp : bp + 64, fold * S + ko + off : fold * S + ko + off + n],
                        start=True,
                        stop=True,
                    )
                    off += n
                ex = expp.tile([128, 1024], BF16, tag="expS")
                nc.scalar.activation(ex[:, :W], ps[:, :W], EXP)
                dd = rcpp.tile([128, 16], BF16, tag="dd")
                df = rcpp.tile([128, 16], F32, tag="df")
                rs = rcpp.tile([128, 16], F32, tag="rs")
                rc = rcpp.tile([128, 16], F32, tag="rc")
                nc.vector.tensor_reduce(
                    dd[:, :nbh],
                    ex[:, :W].rearrange("p (b k) -> p b k", k=64),
                    axis=mybir.AxisListType.X,
                    op=mybir.AluOpType.add,
                )
                mof = 32 * i + (0 if half == 0 else nba)
                # maskc holds 1.0 (in band) / 1e30 (masked): d*1e30 -> rc ~ 0
                nc.vector.tensor_mul(df[:, :nbh], dd[:, :nbh], mkc[:, mof : mof + nbh])
                nc.vector.reciprocal_approx_accurate(rc[:, :nbh], df[:, :nbh], rs[:, :nbh])
                bcast = rc[:, :nbh].unsqueeze(2).broadcast_to([128, nbh, 64])
                k = nrmctr[0]
                nrmctr[0] += 1
                eng = nc.gpsimd if (k % 5) in (0, 3) else nc.vector
                eng.tensor_mul(pt[:, ho : ho + W], ex[:, :W], bcast)
                if debug:
                    u = (i * 4 + h) * 2 + half
                    nc.gpsimd.dma_start(ex_dbg.ap()[:, u * 1024 : u * 1024 + W], ex[:, :W])
                    nc.gpsimd.dma_start(rc_dbg.ap()[:, u * 16 : u * 16 + nbh], rc[:, :nbh])
            # SP-issued transpose guarded by drain + self-copy-DMA fence: the
            # transpose's own DMA-completion sem is unreliable for cross-engine
            # consumers, so after the SP drain (data landed) a 2-byte regular
            # DMA rewrites ptT[0,0] with itself — the AV matmul then waits on
            # that DMACopy's (reliable) completion sem. SP gates no compute,
            # so the drain stall stays off the ACT/DVE critical path.
            nc.sync.dma_start_transpose(
                ptT[:, :WF].rearrange("p (j q) -> p j q", q=128), pt[:, :WF]
            )
            nc.sync.drain()
            nc.sync.dma_start(ptT[0:1, 0:1], ptT[0:1, 0:1])
            return (h, i, acc, ptT, nb, lo)

        def unit_av(pend):
            """AV stage, pipelined one unit behind its transpose."""
            h, i, acc, ptT, nb, lo = pend
            bp = 64 * (h % 2)
            for j in range(nb // 2):
                cp = lo // 2 + j
                nc.tensor.matmul(
                    acc[0:64, 0:128],
                    vv[:, cp * F + h * 64 : cp * F + h * 64 + 64],
                    ptT[:, j * 128 : (j + 1) * 128],
                    start=(j == 0),
                    stop=(j == nb // 2 - 1),
                )
            nc.scalar.copy(
                attT[h // 2][bp : bp + 64, 128 * i : 128 * i + 128], acc[0:64, 0:128]
            )

        def oproj(i):
            for eh in range(2):
                po = psOp.tile([128, 512], F32, tag="psO")
                for p in range(2):
                    nc.tensor.matmul(
                        po[:],
                        attT[p][:, 128 * i : 128 * i + 128],
                        wo[:, p * E + eh * 512 : p * E + eh * 512 + 512],
                        start=(p == 0),
                        stop=(p == 1),
                    )
                ob = outp.tile([128, 512], F32, tag="outsb")
                nc.vector.tensor_copy(ob[:], po[:])
                nc.gpsimd.dma_start(
                    out_d.ap()[128 * i : 128 * i + 128, eh * 512 : (eh + 1) * 512],
                    ob[:],
                )

        pend = None
        for i in range(NCH):
            if i % 4 == 0:
                qproj(i // 4)
            for h in range(HPC):
                nxt = unit(h, i)
                if pend is not None:
                    unit_av(pend)
                pend = nxt
                if h == 2 and i > 0:
                    oproj(i - 1)
        unit_av(pend)
        oproj(NCH - 1)

        if debug:
            nc.gpsimd.dma_start(qT_dbg.ap(), qT[:])
            nc.gpsimd.dma_start(kT_dbg.ap(), kT[:])
            nc.gpsimd.dma_start(vv_dbg.ap(), vv[:])
            nc.gpsimd.dma_start(at_dbg.ap()[:, 0:S], attT[0][:])
            nc.gpsimd.dma_start(at_dbg.ap()[:, S : 2 * S], attT[1][:])

    nc.compile()
    return nc


_NC_CACHE = []


def _get_nc():
    if not _NC_CACHE:
        _NC_CACHE.append(build_nc())
    return _NC_CACHE[0]


def _host_consts():
    # 1.0 for in-band blocks; 1e30 for masked (d*1e30 -> reciprocal ~ 0)
    maskc = np.full((128, NCH * 32), 1e30, np.float32)
    for i in range(NCH):
        for b in range(NBK[i]):
            c = LO[i] + b
            for q2 in range(2):
                r = 2 * i + q2
                if abs(r - c) <= BAND:
                    maskc[q2 * 64 : (q2 + 1) * 64, 32 * i + b] = 1.0
    return (maskc,)


def make_in_maps(query, key, value, Wq, Wk, Wv, Wo):
    bf = ml_dtypes.bfloat16
    (maskc,) = _host_consts()
    in_maps = []
    for c in range(NCORES):
        b, g = divmod(c, HPC)
        fs = slice(F * g, F * (g + 1))
        wo2 = np.concatenate(
            [Wo[:, F * g + 128 * p : F * g + 128 * (p + 1)].T for p in range(2)],
            axis=1,
        )
        in_maps.append(
            {
                "xqT": np.ascontiguousarray(query[b].T).astype(bf),
                "xkT": np.ascontiguousarray(key[b].T).astype(bf),
                "xvT": np.ascontiguousarray(value[b].T).astype(bf),
                "wqT": np.ascontiguousarray((Wq[fs, :] * SCALE).T).astype(bf),
                "wkT": np.ascontiguousarray(Wk[fs, :].T).astype(bf),
                "wvT": np.ascontiguousarray(Wv[fs, :].T).astype(bf),
                "wo2": np.ascontiguousarray(wo2).astype(bf),
                "maskc": maskc,
            }
        )
    return in_maps


def kernel(query, key, value, Wq, Wk, Wv, Wo):
    query = np.asarray(query, np.float32)
    key = np.asarray(key, np.float32)
    value = np.asarray(value, np.float32)
    Wq = np.asarray(Wq, np.float32)
    Wk = np.asarray(Wk, np.float32)
    Wv = np.asarray(Wv, np.float32)
    Wo = np.asarray(Wo, np.float32)

    nc = _get_nc()
    in_maps = make_in_maps(query, key, value, Wq, Wk, Wv, Wo)
    res = bass_utils.run_bass_kernel_spmd(nc, in_maps, core_ids=list(range(NCORES)))
    out = np.zeros((B, S, E), np.float32)
    for c in range(NCORES):
        b = c // HPC
        out[b] += res.results[c]["out"]
    return out
